# revision 11
# baseline (speedup 1.0000x reference)
"""Trainium2 Bass kernel for nn_MACRO_VRNN (per-agent VRNN, 10 agents, 64 steps).

Strategy (8 cores, uniform SPMD program):
  - core c runs the full-batch recurrence chain of agent c (B=128) plus a
    quarter-batch chain of agent 8 + c//4 (batch slice 32*(c%4) .. +32).
  - Phase 1 (sequential): encoder + GRU per step; z/mean/std/h stored to DRAM
    scratch in bf16.
  - Phase 2 (batched): prior + decoder + loss terms over all stored steps with
    512-column streaming matmuls.
  - Each core emits 5 partial sums; the host folds constants and reduces.

All matmuls bf16 (fp32 PSUM). Only ACT functions used: Exp / Ln / Identity-free
ops (single table set: natural_log_exp_and_others).
  softplus(x) = ln(exp(x)+1);  sigmoid(v) = 1/(1+exp(-v));
  tanh(v) = 1 - 2/(1+exp(2v)).
"""

import numpy as np
import ml_dtypes

BF = ml_dtypes.bfloat16

# problem dims (hardcoded per contest contract)
A, T, B = 10, 65, 128
NT = T - 1
XD, YD, ZD, HD, MD, RM = 2, 20, 64, 256, 90, 256
QB = 32                     # quarter-batch size
NFA = NT * B                # 8192 columns, full chain
NQA = NT * QB               # 2048 columns, quarter chain
LOG2PI = float(np.log(2.0 * np.pi))
NB = 11                     # bias columns

_ENC_IN = XD + MD + RM      # 348
_PRI_IN = MD + RM           # 346
_DEC_IN = YD + MD + ZD + RM # 430
_GRU_IN = XD + ZD           # 66


def _wlayout():
    """Weight tile layout inside the packed [128, WCOLS] per-agent matrix.
    Returns dict name -> (col_off, K) plus total cols. All tiles are sliced in
    128-col M-chunks at matmul time, except encms/prims (128) and decms (34)."""
    lay = {}
    off = 0

    def add(name, K, M):
        nonlocal off
        lay[name] = (off, K)
        off += M

    add('enc1_k0', 93, 256)   # rows: [W1[2:92](m), enc_b1, W1[0:2](x)]
    add('enc1_k1', 128, 256)
    add('enc1_k2', 128, 256)
    add('enc2_k0', 128, 256)
    add('enc2_k1', 128, 256)
    add('encms_k0', 128, 128)  # M: [encm | encs]
    add('encms_k1', 128, 128)
    add('pri1_k0', 91, 256)   # rows: [P1[0:90](m), pri_b1]
    add('pri1_k1', 128, 256)
    add('pri1_k2', 128, 256)
    add('pri2_k0', 128, 256)
    add('pri2_k1', 128, 256)
    add('prims_k0', 128, 128)
    add('prims_k1', 128, 128)
    add('dec1_k0', 111, 256)  # rows: [D1[20:110](m), D1[0:20](y), dec_b1]
    add('dec1_k1', 64, 256)   # z rows D1[110:174]
    add('dec1_k2', 128, 256)
    add('dec1_k3', 128, 256)
    add('dec2_k0', 128, 256)
    add('dec2_k1', 128, 256)
    add('decms_k0', 128, 34)  # M: [decm(0:2), zeros, decs(32:34)]
    add('decms_k1', 128, 34)
    add('grz_k0', 67, 512)    # rows: [Wih[2:66](z), Wih[0:2](x), bih+bhh][:,0:512]
    add('grz_k1', 128, 512)
    add('grz_k2', 128, 512)
    add('gn_ih', 67, 256)     # rows: [Wih z, Wih x, bih][:,512:768]
    add('gn_h0', 128, 256)
    add('gn_h1', 128, 256)
    return lay, off


_LAY, WCOLS = _wlayout()


def _pack_w(inp, a):
    """Pack all weights of agent a into [128, WCOLS] bf16."""
    out = np.zeros((128, WCOLS), np.float32)

    def put(name, rows):
        off, K = _LAY[name]
        assert rows.shape[0] == K, (name, rows.shape)
        out[:K, off:off + rows.shape[1]] = rows

    W1 = inp['enc_W1'][a]
    put('enc1_k0', np.concatenate([W1[2:92], inp['enc_b1'][a][None, :], W1[0:2]], 0))
    put('enc1_k1', W1[92:220])
    put('enc1_k2', W1[220:348])
    W2 = inp['enc_W2'][a]
    put('enc2_k0', W2[0:128]); put('enc2_k1', W2[128:256])
    Wms = np.concatenate([inp['encm_W'][a], inp['encs_W'][a]], 1)  # [256,128]
    put('encms_k0', Wms[0:128]); put('encms_k1', Wms[128:256])

    P1 = inp['pri_W1'][a]
    put('pri1_k0', np.concatenate([P1[0:90], inp['pri_b1'][a][None, :]], 0))
    put('pri1_k1', P1[90:218]); put('pri1_k2', P1[218:346])
    P2 = inp['pri_W2'][a]
    put('pri2_k0', P2[0:128]); put('pri2_k1', P2[128:256])
    Pms = np.concatenate([inp['prim_W'][a], inp['pris_W'][a]], 1)
    put('prims_k0', Pms[0:128]); put('prims_k1', Pms[128:256])

    D1 = inp['dec_W1'][a]
    put('dec1_k0', np.concatenate([D1[20:110], D1[0:20], inp['dec_b1'][a][None, :]], 0))
    put('dec1_k1', D1[110:174])
    put('dec1_k2', D1[174:302]); put('dec1_k3', D1[302:430])
    D2 = inp['dec_W2'][a]
    put('dec2_k0', D2[0:128]); put('dec2_k1', D2[128:256])
    Dms = np.zeros((256, 34), np.float32)
    Dms[:, 0:2] = inp['decm_W'][a]
    Dms[:, 32:34] = inp['decs_W'][a]
    put('decms_k0', Dms[0:128]); put('decms_k1', Dms[128:256])

    Wih = inp['gru_Wih'][a]   # [66, 768]
    Whh = inp['gru_Whh'][a]   # [256, 768]
    brz = (inp['gru_bih'][a] + inp['gru_bhh'][a])[0:512]
    put('grz_k0', np.concatenate([Wih[2:66, 0:512], Wih[0:2, 0:512], brz[None, :]], 0))
    put('grz_k1', Whh[0:128, 0:512]); put('grz_k2', Whh[128:256, 0:512])
    put('gn_ih', np.concatenate([Wih[2:66, 512:768], Wih[0:2, 512:768],
                                 inp['gru_bih'][a][None, 512:768]], 0))
    put('gn_h0', Whh[0:128, 512:768]); put('gn_h1', Whh[128:256, 512:768])
    return out.astype(BF)


def _pack_b(inp, a):
    """Non-weight-carried biases: [128, NB] f32."""
    out = np.zeros((128, NB), np.float32)
    out[:, 0] = inp['enc_b2'][a][0:128]
    out[:, 1] = inp['enc_b2'][a][128:256]
    out[0:64, 2] = inp['encm_b'][a]
    out[64:128, 2] = inp['encs_b'][a]
    out[:, 3] = inp['pri_b2'][a][0:128]
    out[:, 4] = inp['pri_b2'][a][128:256]
    out[0:64, 5] = inp['prim_b'][a]
    out[64:128, 5] = inp['pris_b'][a]
    out[:, 6] = inp['dec_b2'][a][0:128]
    out[:, 7] = inp['dec_b2'][a][128:256]
    out[0:2, 8] = inp['decm_b'][a]
    out[32:34, 8] = inp['decs_b'][a]
    out[:, 9] = inp['gru_bhh'][a][512:640]
    out[:, 10] = inp['gru_bhh'][a][640:768]
    return out


# ---------------------------------------------------------------------------
# device program
# ---------------------------------------------------------------------------

def build_program(nt=NT):
    import concourse.bass as bass
    import concourse.tile as tile
    from concourse import bacc, mybir
    from contextlib import ExitStack

    f32 = mybir.dt.float32
    bf16 = mybir.dt.bfloat16
    AF = mybir.ActivationFunctionType
    ALU = mybir.AluOpType
    AX = mybir.AxisListType

    nfa = nt * B
    nqa = nt * QB

    nc = bacc.Bacc("TRN2", target_bir_lowering=False, debug=False, num_devices=8)

    def dram(name, shape, dt, kind):
        return nc.dram_tensor(name, shape, dt, kind=kind).ap()

    w_fa = dram('w_fa', (128, WCOLS), bf16, 'ExternalInput')
    w_qa = dram('w_qa', (128, WCOLS), bf16, 'ExternalInput')
    bi_fa = dram('bi_fa', (128, NB), f32, 'ExternalInput')
    bi_qa = dram('bi_qa', (128, NB), f32, 'ExternalInput')
    dT16 = dram('dT16', (T, YD, B), bf16, 'ExternalInput')
    dT32 = dram('dT32', (T, YD, B), f32, 'ExternalInput')
    mac_fa = dram('mac_fa', (nfa,), bf16, 'ExternalInput')
    mac_qa = dram('mac_qa', (nqa,), bf16, 'ExternalInput')
    eps_fa = dram('eps_fa', (ZD + 2, nt, B), bf16, 'ExternalInput')
    eps_qa = dram('eps_qa', (ZD + 2, nt, QB), bf16, 'ExternalInput')
    idx = dram('idx', (128, 1), f32, 'ExternalInput')
    onesr = dram('onesr', (B,), bf16, 'ExternalInput')
    xsel = dram('xsel', (2, 2), f32, 'ExternalInput')  # per-core agent x-offsets: [[off_fa],[off_qa]] packed
    partials = dram('partials', (8, 1), f32, 'ExternalOutput')

    sms_fa = dram('sms_fa', (nt, 3, ZD, B), bf16, 'Internal')
    sms_qa = dram('sms_qa', (nt, 3, ZD, QB), bf16, 'Internal')
    sh_fa = dram('sh_fa', (nt, 2, 128, B), bf16, 'Internal')
    sh_qa = dram('sh_qa', (nt, 2, 128, QB), bf16, 'Internal')

    class Chain:
        pass

    with tile.TileContext(nc) as tc, ExitStack() as ctx:
        singles = ctx.enter_context(tc.tile_pool(name="singles", bufs=1))
        proto = ctx.enter_context(tc.tile_pool(name="proto", bufs=1))
        p1w = ctx.enter_context(tc.tile_pool(name="p1w", bufs=3))
        p2w = ctx.enter_context(tc.tile_pool(name="p2w", bufs=2))

        chains = []
        for ci, (wap, bap, mac, epsd, Bl, N, sms, sh) in enumerate([
            (w_fa, bi_fa, mac_fa, eps_fa, B, nfa, sms_fa, sh_fa),
            (w_qa, bi_qa, mac_qa, eps_qa, QB, nqa, sms_qa, sh_qa),
        ]):
            ch = Chain()
            ch.i, ch.Bl, ch.N, ch.sms, ch.sh = ci, Bl, N, sms, sh
            ch.W = singles.tile([128, WCOLS], bf16, tag=f'W{ci}')
            nc.sync.dma_start(out=ch.W, in_=wap)
            ch.Bi = singles.tile([128, NB], f32, tag=f'Bi{ci}')
            nc.sync.dma_start(out=ch.Bi, in_=bap)
            ch.epsd = epsd
            # Mx: [m(0:90) onehot, ones(90), x(91:93)]
            ch.Mx = singles.tile([93, N], bf16, tag=f'Mx{ci}')
            mb = proto.tile([90, N], bf16, tag=f'mb{ci}')
            nc.sync.dma_start(out=mb, in_=bass.AP(tensor=mac.tensor, offset=0,
                                                  ap=[[0, 90], [1, N]]))
            if ci == 0:
                idx_sb = singles.tile([128, 1], f32, tag='idx')
                nc.sync.dma_start(out=idx_sb, in_=idx)
            nc.vector.tensor_scalar(ch.Mx[0:90, :], mb, idx_sb[0:90, :], None,
                                    op0=ALU.is_equal)
            nc.sync.dma_start(out=ch.Mx[90:91, :],
                              in_=bass.AP(tensor=onesr.tensor, offset=0,
                                          ap=[[0, 1], [0, nt], [1, Bl]]))
            # x rows via host-shifted dT16 copies: separate per-core input would
            # need agent-dependent offset; instead host packs x rows directly:
            ch.xr = dram(f'xr{ci}', (2, nt, Bl), bf16, 'ExternalInput')
            nc.sync.dma_start(out=ch.Mx[91:93, :].rearrange("p (t b) -> p t b", b=Bl),
                              in_=ch.xr)
            # persistent zx: [z(0:64), x(64:66), ones(66)]
            ch.zx = singles.tile([67, Bl], bf16, tag=f'zx{ci}')
            nc.sync.dma_start(out=ch.zx[66:67, :],
                              in_=bass.AP(tensor=onesr.tensor, offset=0,
                                          ap=[[0, 1], [1, Bl]]))
            # h state
            ch.h = singles.tile([128, 2, Bl], bf16, tag=f'h{ci}')
            nc.vector.memset(ch.h, 0.0)
            chains.append(ch)

        def wsl(ch, name, m, Mw=128):
            off, K = _LAY[name]
            return ch.W[0:K, off + m * Mw: off + (m + 1) * Mw]

        # ------------------------------------------------------------------
        # phase 1
        # ------------------------------------------------------------------
        with tc.tile_pool(name="p1psum", bufs=1, space="PSUM") as p1psum:
            for t in range(nt):
                for ch in chains:
                    ci, Bl = ch.i, ch.Bl
                    Bi = ch.Bi
                    mxs = ch.Mx[:, t * Bl:(t + 1) * Bl]
                    pe = p1psum.tile([128, 2, Bl], f32, tag=f'pe{ci}', bufs=2)
                    for m in (0, 1):
                        nc.tensor.matmul(pe[:, m, :], wsl(ch, 'enc1_k0', m), mxs[0:93, :],
                                         start=True, stop=False)
                        nc.tensor.matmul(pe[:, m, :], wsl(ch, 'enc1_k1', m), ch.h[:, 0, :],
                                         start=False, stop=False)
                        nc.tensor.matmul(pe[:, m, :], wsl(ch, 'enc1_k2', m), ch.h[:, 1, :],
                                         start=False, stop=True)
                    e1 = p1w.tile([128, 2, Bl], bf16, tag=f'e1{ci}')
                    nc.vector.tensor_scalar(e1, pe, 0.0, None, op0=ALU.max)
                    pe2 = p1psum.tile([128, 2, Bl], f32, tag=f'pe{ci}', bufs=2)
                    for m in (0, 1):
                        nc.tensor.matmul(pe2[:, m, :], wsl(ch, 'enc2_k0', m), e1[:, 0, :],
                                         start=True, stop=False)
                        nc.tensor.matmul(pe2[:, m, :], wsl(ch, 'enc2_k1', m), e1[:, 1, :],
                                         start=False, stop=True)
                    e2 = p1w.tile([128, 2, Bl], bf16, tag=f'e2{ci}')
                    for m in (0, 1):
                        nc.vector.tensor_scalar(e2[:, m, :], pe2[:, m, :], Bi[:, m:m + 1],
                                                0.0, op0=ALU.add, op1=ALU.max)
                    pms = p1psum.tile([128, Bl], f32, tag=f'pe{ci}', bufs=2)
                    for k in (0, 1):
                        nc.tensor.matmul(pms, wsl(ch, f'encms_k{k}', 0), e2[:, k, :],
                                         start=(k == 0), stop=(k == 1))
                    stg = p1w.tile([ZD, 3, Bl], bf16, tag=f'stg{ci}')
                    nc.vector.tensor_scalar(stg[:, 0, :], pms[0:64, :], Bi[0:64, 2:3],
                                            None, op0=ALU.add)
                    ese = p1w.tile([ZD, Bl], f32, tag=f'ese{ci}')
                    nc.scalar.activation(ese, pms[64:128, :], AF.Exp,
                                         bias=Bi[64:128, 2:3], scale=1.0)
                    nc.scalar.activation(stg[:, 1, :], ese, AF.Ln, bias=1.0, scale=1.0)
                    epst = p1w.tile([ZD + 2, Bl], bf16, tag=f'epst{ci}')
                    nc.sync.dma_start(out=epst,
                                      in_=bass.AP(tensor=ch.epsd.tensor, offset=t * Bl,
                                                  ap=[[nt * Bl, ZD + 2], [1, Bl]]))
                    nc.vector.tensor_tensor(ch.zx[0:64, :], epst[0:64, :], stg[:, 1, :],
                                            op=ALU.mult)
                    nc.vector.tensor_tensor(ch.zx[0:64, :], ch.zx[0:64, :], stg[:, 0, :],
                                            op=ALU.add)
                    nc.gpsimd.tensor_copy(stg[:, 2, :], ch.zx[0:64, :])
                    nc.gpsimd.tensor_copy(ch.zx[64:66, :], epst[64:66, :])
                    # store h_t (pre-update) and stg
                    nc.sync.dma_start(
                        out=bass.AP(tensor=ch.sh.tensor, offset=t * 2 * 128 * Bl,
                                    ap=[[Bl, 128], [128 * Bl, 2], [1, Bl]]),
                        in_=ch.h)
                    nc.sync.dma_start(
                        out=bass.AP(tensor=ch.sms.tensor, offset=t * 3 * ZD * Bl,
                                    ap=[[Bl, ZD], [ZD * Bl, 3], [1, Bl]]),
                        in_=stg)
                    # gru
                    pg = p1psum.tile([128, 8, Bl], f32, tag=f'pg{ci}', bufs=1)
                    for m in range(4):
                        nc.tensor.matmul(pg[:, m, :], wsl(ch, 'grz_k0', m), ch.zx[0:67, :],
                                         start=True, stop=False)
                        nc.tensor.matmul(pg[:, m, :], wsl(ch, 'grz_k1', m), ch.h[:, 0, :],
                                         start=False, stop=False)
                        nc.tensor.matmul(pg[:, m, :], wsl(ch, 'grz_k2', m), ch.h[:, 1, :],
                                         start=False, stop=True)
                    for m in (0, 1):
                        nc.tensor.matmul(pg[:, 4 + m, :], wsl(ch, 'gn_ih', m), ch.zx[0:67, :],
                                         start=True, stop=True)
                        nc.tensor.matmul(pg[:, 6 + m, :], wsl(ch, 'gn_h0', m), ch.h[:, 0, :],
                                         start=True, stop=False)
                        nc.tensor.matmul(pg[:, 6 + m, :], wsl(ch, 'gn_h1', m), ch.h[:, 1, :],
                                         start=False, stop=True)
                    rze = p1w.tile([128, 4, Bl], bf16, tag=f'rze{ci}')
                    nc.scalar.activation(rze, pg[:, 0:4, :], AF.Exp, bias=0.0, scale=-1.0)
                    nc.gpsimd.tensor_scalar_add(rze, rze, 1.0)
                    rz = p1w.tile([128, 4, Bl], bf16, tag=f'rz{ci}')
                    with nc.allow_low_precision(reason="bf16 sigmoid gates"):
                        nc.vector.reciprocal(rz, rze)
                    ntmp = p1w.tile([128, 2, Bl], f32, tag=f'nt{ci}')
                    for m in (0, 1):
                        nc.vector.scalar_tensor_tensor(ntmp[:, m, :], pg[:, 6 + m, :],
                                                       Bi[:, 9 + m:10 + m], rz[:, m, :],
                                                       op0=ALU.add, op1=ALU.mult)
                    npre = p1w.tile([128, 2, Bl], f32, tag=f'np{ci}')
                    nc.vector.tensor_tensor(npre, ntmp, pg[:, 4:6, :], op=ALU.add)
                    ne = p1w.tile([128, 2, Bl], bf16, tag=f'ne{ci}')
                    nc.scalar.activation(ne, npre, AF.Exp, bias=0.0, scale=2.0)
                    nc.gpsimd.tensor_scalar_add(ne, ne, 1.0)
                    nn_ = p1w.tile([128, 2, Bl], bf16, tag=f'nn{ci}')
                    with nc.allow_low_precision(reason="bf16 tanh gate"):
                        nc.vector.reciprocal(nn_, ne)
                    nc.vector.tensor_scalar(nn_, nn_, -2.0, 1.0, op0=ALU.mult, op1=ALU.add)
                    hd = p1w.tile([128, 2, Bl], bf16, tag=f'hd{ci}')
                    nc.vector.tensor_tensor(hd, ch.h, nn_, op=ALU.subtract)
                    nc.vector.tensor_tensor(hd, rz[:, 2:4, :], hd, op=ALU.mult)
                    nc.vector.tensor_tensor(ch.h, hd, nn_, op=ALU.add)

        # ------------------------------------------------------------------
        # phase 2
        # ------------------------------------------------------------------
        acc_lps = singles.tile([ZD, 512], f32, tag='acc_lps')
        acc_les = singles.tile([ZD, 512], f32, tag='acc_les')
        acc_q = singles.tile([ZD, 512], f32, tag='acc_q')
        acc_r1 = singles.tile([2, 512], f32, tag='acc_r1')
        acc_r2 = singles.tile([34, 512], f32, tag='acc_r2')
        for acc in (acc_lps, acc_les, acc_q, acc_r1, acc_r2):
            nc.vector.memset(acc, 0.0)

        with tc.tile_pool(name="p2psum", bufs=1, space="PSUM") as p2psum:
            for ch in chains:
                ci, Bl, N = ch.i, ch.Bl, ch.N
                Bi = ch.Bi
                ntc = 4 if ci == 0 else 16
                ntc = min(ntc, nt)
                C = ntc * Bl
                nch = (nt + ntc - 1) // ntc
                # YM: [m(0:90), y(90:110), ones(110)]
                YM = singles.tile([111, N], bf16, tag=f'YM{ci}')
                nc.vector.tensor_copy(YM[0:90, :], ch.Mx[0:90, :])
                nc.sync.dma_start(out=YM[110:111, :],
                                  in_=bass.AP(tensor=onesr.tensor, offset=0,
                                              ap=[[0, 1], [0, nt], [1, Bl]]))
                ch.yr = dram(f'yr{ci}', (YD, nt, Bl), bf16, 'ExternalInput')
                nc.sync.dma_start(out=YM[90:110, :].rearrange("p (t b) -> p t b", b=Bl),
                                  in_=ch.yr)
                ch.x32d = dram(f'x32_{ci}', (2, nt, Bl), f32, 'ExternalInput')

                for cc in range(nch):
                    t0 = cc * ntc
                    cols = slice(t0 * Bl, (t0 + ntc) * Bl)
                    smsb = p2w.tile([ZD, 3, ntc, Bl], bf16, tag='sms')
                    for j in range(3):
                        nc.sync.dma_start(
                            out=smsb[:, j, :, :],
                            in_=bass.AP(tensor=ch.sms.tensor,
                                        offset=(t0 * 3 + j) * ZD * Bl,
                                        ap=[[Bl, ZD], [3 * ZD * Bl, ntc], [1, Bl]]))
                    hb = p2w.tile([128, 2, ntc, Bl], bf16, tag='hb')
                    for k in range(2):
                        nc.sync.dma_start(
                            out=hb[:, k, :, :],
                            in_=bass.AP(tensor=ch.sh.tensor,
                                        offset=(t0 * 2 + k) * 128 * Bl,
                                        ap=[[Bl, 128], [2 * 128 * Bl, ntc], [1, Bl]]))
                    x32 = p2w.tile([2, ntc, Bl], f32, tag='x32')
                    nc.sync.dma_start(
                        out=x32,
                        in_=bass.AP(tensor=ch.x32d.tensor, offset=t0 * Bl,
                                    ap=[[nt * Bl, 2], [Bl, ntc], [1, Bl]]))
                    em = smsb[:, 0, :, :].rearrange("p a b -> p (a b)")
                    es = smsb[:, 1, :, :].rearrange("p a b -> p (a b)")
                    zc = smsb[:, 2, :, :].rearrange("p a b -> p (a b)")
                    x32f = x32.rearrange("p a b -> p (a b)")

                    # --- prior ---
                    pp = p2psum.tile([128, 2, C], f32, tag='pp', bufs=2)
                    for m in (0, 1):
                        nc.tensor.matmul(pp[:, m, :], wsl(ch, 'pri1_k0', m), ch.Mx[0:91, cols],
                                         start=True, stop=False)
                        nc.tensor.matmul(pp[:, m, :], wsl(ch, 'pri1_k1', m),
                                         hb[:, 0, :, :].rearrange("p a b -> p (a b)"),
                                         start=False, stop=False)
                        nc.tensor.matmul(pp[:, m, :], wsl(ch, 'pri1_k2', m),
                                         hb[:, 1, :, :].rearrange("p a b -> p (a b)"),
                                         start=False, stop=True)
                    p1t = p2w.tile([128, 2, C], bf16, tag='mlp')
                    nc.vector.tensor_scalar(p1t, pp, 0.0, None, op0=ALU.max)
                    pp2 = p2psum.tile([128, 2, C], f32, tag='pp', bufs=2)
                    for m in (0, 1):
                        nc.tensor.matmul(pp2[:, m, :], wsl(ch, 'pri2_k0', m), p1t[:, 0, :],
                                         start=True, stop=False)
                        nc.tensor.matmul(pp2[:, m, :], wsl(ch, 'pri2_k1', m), p1t[:, 1, :],
                                         start=False, stop=True)
                    p2t = p2w.tile([128, 2, C], bf16, tag='mlp')
                    for m in (0, 1):
                        nc.vector.tensor_scalar(p2t[:, m, :], pp2[:, m, :], Bi[:, 3 + m:4 + m],
                                                0.0, op0=ALU.add, op1=ALU.max)
                    pms = p2psum.tile([128, C], f32, tag='pms', bufs=2)
                    for k in (0, 1):
                        nc.tensor.matmul(pms, wsl(ch, f'prims_k{k}', 0), p2t[:, k, :],
                                         start=(k == 0), stop=(k == 1))
                    # KL pieces
                    pse = p2w.tile([ZD, C], f32, tag='pse')
                    nc.scalar.activation(pse, pms[64:128, :], AF.Exp,
                                         bias=Bi[64:128, 5:6], scale=1.0)
                    ps = p2w.tile([ZD, C], f32, tag='ps')
                    nc.scalar.activation(ps, pse, AF.Ln, bias=1.0, scale=1.0)
                    lps = p2w.tile([ZD, C], f32, tag='ltmp')
                    nc.scalar.activation(lps, ps, AF.Ln, bias=0.0, scale=1.0)
                    nc.vector.tensor_tensor(acc_lps[:, 0:C], acc_lps[:, 0:C], lps, op=ALU.add)
                    les = p2w.tile([ZD, C], f32, tag='ltmp')
                    nc.scalar.activation(les, es, AF.Ln, bias=0.0, scale=1.0)
                    nc.vector.tensor_tensor(acc_les[:, 0:C], acc_les[:, 0:C], les, op=ALU.add)
                    ndm = p2w.tile([ZD, C], f32, tag='ndm')
                    nc.vector.scalar_tensor_tensor(ndm, pms[0:64, :], Bi[0:64, 5:6], em,
                                                   op0=ALU.add, op1=ALU.subtract)
                    es2 = p2w.tile([ZD, C], f32, tag='es2')
                    nc.gpsimd.tensor_tensor(es2, es, es, op=ALU.mult)
                    rp = p2w.tile([ZD, C], f32, tag='rp')
                    nc.vector.reciprocal(rp, ps)
                    rp2 = p2w.tile([ZD, C], f32, tag=f'rp2{ci}')
                    nc.gpsimd.tensor_tensor(rp2, rp, rp, op=ALU.mult)
                    dm2 = p2w.tile([ZD, C], f32, tag='dm2')
                    nc.vector.tensor_tensor(dm2, ndm, ndm, op=ALU.mult)
                    nc.vector.tensor_tensor(dm2, dm2, es2, op=ALU.add)
                    nc.vector.tensor_tensor(dm2, dm2, rp2, op=ALU.mult)
                    nc.vector.tensor_tensor(acc_q[:, 0:C], acc_q[:, 0:C], dm2, op=ALU.add)

                    # --- decoder ---
                    ppd = p2psum.tile([128, 2, C], f32, tag='pp', bufs=2)
                    for m in (0, 1):
                        nc.tensor.matmul(ppd[:, m, :], wsl(ch, 'dec1_k0', m), YM[0:111, cols],
                                         start=True, stop=False)
                        nc.tensor.matmul(ppd[:, m, :], wsl(ch, 'dec1_k1', m), zc,
                                         start=False, stop=False)
                        nc.tensor.matmul(ppd[:, m, :], wsl(ch, 'dec1_k2', m),
                                         hb[:, 0, :, :].rearrange("p a b -> p (a b)"),
                                         start=False, stop=False)
                        nc.tensor.matmul(ppd[:, m, :], wsl(ch, 'dec1_k3', m),
                                         hb[:, 1, :, :].rearrange("p a b -> p (a b)"),
                                         start=False, stop=True)
                    d1t = p2w.tile([128, 2, C], bf16, tag='mlp')
                    nc.vector.tensor_scalar(d1t, ppd, 0.0, None, op0=ALU.max)
                    ppd2 = p2psum.tile([128, 2, C], f32, tag='pp', bufs=2)
                    for m in (0, 1):
                        nc.tensor.matmul(ppd2[:, m, :], wsl(ch, 'dec2_k0', m), d1t[:, 0, :],
                                         start=True, stop=False)
                        nc.tensor.matmul(ppd2[:, m, :], wsl(ch, 'dec2_k1', m), d1t[:, 1, :],
                                         start=False, stop=True)
                    d2t = p2w.tile([128, 2, C], bf16, tag='mlp')
                    for m in (0, 1):
                        nc.vector.tensor_scalar(d2t[:, m, :], ppd2[:, m, :], Bi[:, 6 + m:7 + m],
                                                0.0, op0=ALU.add, op1=ALU.max)
                    pdm = p2psum.tile([34, C], f32, tag='pdm', bufs=2)
                    for k in (0, 1):
                        nc.tensor.matmul(pdm, wsl(ch, f'decms_k{k}', 0, 34), d2t[:, k, :],
                                         start=(k == 0), stop=(k == 1))
                    dse = p2w.tile([34, C], f32, tag='dtmp')
                    nc.scalar.activation(dse[32:34, :], pdm[32:34, :], AF.Exp,
                                         bias=Bi[32:34, 8:9], scale=1.0)
                    dsl = p2w.tile([34, C], f32, tag='dsl')
                    nc.scalar.activation(dsl[32:34, :], dse[32:34, :], AF.Ln, bias=1.0, scale=1.0)
                    ldl = p2w.tile([34, C], f32, tag='dtmp')
                    nc.scalar.activation(ldl[32:34, :], dsl[32:34, :], AF.Ln, bias=0.0, scale=1.0)
                    nc.vector.tensor_tensor(acc_r2[32:34, 0:C], acc_r2[32:34, 0:C],
                                            ldl[32:34, :], op=ALU.add)
                    rds = p2w.tile([2, C], f32, tag='rds')
                    nc.vector.reciprocal(rds, dsl[32:34, :])
                    nd = p2w.tile([2, C], f32, tag='nd')
                    nc.vector.scalar_tensor_tensor(nd, pdm[0:2, :], Bi[0:2, 8:9], x32f,
                                                   op0=ALU.add, op1=ALU.subtract)
                    nc.vector.tensor_tensor(nd, nd, rds, op=ALU.mult)
                    sq = p2w.tile([2, C], f32, tag='sq')
                    nc.gpsimd.tensor_tensor(sq, nd, nd, op=ALU.mult)
                    nc.vector.scalar_tensor_tensor(acc_r1[:, 0:C], sq, 0.5, acc_r1[:, 0:C],
                                                   op0=ALU.mult, op1=ALU.add)

            # final reduction
            red = singles.tile([128, 8], f32, tag='red')
            nc.vector.memset(red, 0.0)
            nc.vector.tensor_reduce(red[0:64, 0:1], acc_lps, axis=AX.X, op=ALU.add)
            nc.vector.tensor_reduce(red[0:64, 1:2], acc_les, axis=AX.X, op=ALU.add)
            nc.vector.tensor_reduce(red[0:64, 2:3], acc_q, axis=AX.X, op=ALU.add)
            nc.vector.tensor_reduce(red[0:2, 3:4], acc_r1, axis=AX.X, op=ALU.add)
            nc.vector.tensor_reduce(red[0:34, 4:5], acc_r2, axis=AX.X, op=ALU.add)
            mask = singles.tile([128, 1], f32, tag='mask')
            nc.vector.memset(mask, 1.0)
            ppp = p2psum.tile([8, 1], f32, tag='pms', bufs=2)
            nc.tensor.matmul(ppp, red, mask, start=True, stop=True)
            psb = singles.tile([8, 1], f32, tag='psb')
            nc.vector.tensor_copy(psb, ppp)
            nc.sync.dma_start(out=partials, in_=psb)

    nc.compile()
    return nc


# ---------------------------------------------------------------------------
# host wrapper
# ---------------------------------------------------------------------------

_CACHE = {}


def make_in_maps(inputs, nt=NT):
    """Returns list of 8 per-core input dicts."""
    data = np.asarray(inputs['data'], np.float32)
    macro = np.asarray(inputs['macro'])
    eps = np.asarray(inputs['eps'], np.float32)
    dataT = np.ascontiguousarray(data.transpose(0, 2, 1))      # [T, 20, B]
    dT16 = dataT.astype(BF)
    idx_np = np.arange(128, dtype=np.float32).reshape(128, 1)
    ones_np = np.ones((B,), BF)

    w_cache = {a: _pack_w(inputs, a) for a in range(A)}
    b_cache = {a: _pack_b(inputs, a) for a in range(A)}

    in_maps = []
    for c in range(8):
        fa, qa, qs = c, 8 + c // 4, QB * (c % 4)
        m = {
            'w_fa': w_cache[fa], 'w_qa': w_cache[qa],
            'bi_fa': b_cache[fa], 'bi_qa': b_cache[qa],
            'dT16': dT16, 'dT32': dataT,
            'idx': idx_np, 'onesr': ones_np,
            'xsel': np.zeros((2, 2), np.float32),
        }
        for ci, (a, b0, Bl) in enumerate([(fa, 0, B), (qa, qs, QB)]):
            mac = macro[:nt, b0:b0 + Bl, a].astype(np.float32).astype(BF).reshape(-1)
            ep = np.ascontiguousarray(
                eps[:nt, a, b0:b0 + Bl, :].transpose(2, 0, 1))  # [zd, t, b]
            xrow = dataT[1:nt + 1, 2 * a:2 * a + 2, b0:b0 + Bl].transpose(1, 0, 2)
            ep = np.concatenate([ep, xrow], axis=0).astype(BF)  # [zd+2, t, b]
            xr = np.ascontiguousarray(
                dataT[1:nt + 1, 2 * a:2 * a + 2, b0:b0 + Bl].transpose(1, 0, 2)).astype(BF)
            yr = np.ascontiguousarray(
                dataT[0:nt, :, b0:b0 + Bl].transpose(1, 0, 2)).astype(BF)
            x32 = np.ascontiguousarray(
                dataT[1:nt + 1, 2 * a:2 * a + 2, b0:b0 + Bl].transpose(1, 0, 2))
            sfx = 'fa' if ci == 0 else 'qa'
            m[f'mac_{sfx}'] = mac
            m[f'eps_{sfx}'] = ep
            m[f'xr{ci}'] = xr
            m[f'yr{ci}'] = yr
            m[f'x32_{ci}'] = x32
        in_maps.append(m)
    return in_maps


def combine_partials(parts, nt=NT):
    """parts: list of 8 arrays (8,1) f32 -> (recon, kl) float32."""
    S1 = S2 = S3 = R1 = R2 = 0.0
    for p in parts:
        p = np.asarray(p, np.float64).reshape(-1)
        S1 += p[0]; S2 += p[1]; S3 += p[2]; R1 += p[3]; R2 += p[4]
    NKL = nt * A * B * ZD
    NREC = nt * A * B * XD
    kl = S1 - S2 + 0.5 * S3 - 0.5 * NKL
    recon = R1 + R2 + 0.5 * LOG2PI * NREC
    return np.float32(recon), np.float32(kl)


def kernel(**inputs):
    from concourse.bass_utils import run_bass_kernel_spmd
    if 'nc' not in _CACHE:
        _CACHE['nc'] = build_program(NT)
    nc = _CACHE['nc']
    in_maps = make_in_maps(inputs, NT)
    res = run_bass_kernel_spmd(nc, in_maps, core_ids=list(range(8)))
    parts = [r['partials'] for r in res.results]
    recon, kl = combine_partials(parts, NT)
    return np.array([recon, kl], dtype=np.float32)
